# revision 1
# baseline (speedup 1.0000x reference)
"""MoE layer (top-2 routing, 8 experts) on 8 Trainium2 NeuronCores.

Strategy — expert-parallel with hidden-dim (H) slicing for perfect balance:
  - Host computes the gate (router math in fp64 numpy): logits, top-2 experts
    per token, softmax gates; tokens are sorted into per-expert segments.
  - ReLU is elementwise in H, so each expert MLP decomposes exactly into 8
    independent H-slice MLPs (D x 512 x D). Core c holds slice c of EVERY
    expert (same 16.8MB fp16 weight footprint as one whole expert).
  - The kernel runs 8 passes; pass e = all 8 cores compute expert e's slice
    over exactly n_e tokens (identical shapes on every core -> SPMD, zero
    padding, perfect load balance).
  - Each core emits gate-weighted partial outputs; host sums the 8 cores'
    partials and scatter-adds each token's two expert contributions.

Hardcoded problem shape: x(8192,1024) w1(8,1024,4096) w2(8,4096,1024).
"""

import numpy as np

import concourse.tile as tile
import concourse.mybir as mybir
from concourse import bacc
from concourse.bass_utils import run_bass_kernel_spmd

E = 8          # experts
D = 1024       # model dim
H = 4096       # hidden dim
HS = H // 8    # per-core hidden slice (512)
NHS = HS // 128  # h-tiles per slice (4)
TOP_K = 2
N_CORES = 8
ND = D // 128   # 8 d-tiles

F32 = mybir.dt.float32
F16 = mybir.dt.float16


def _balanced_tiles(start, n, max_tile=512):
    """Split [start, start+n) into ceil(n/max_tile) near-equal tiles."""
    nt = max(1, -(-n // max_tile))
    base, rem = divmod(n, nt)
    tiles = []
    t = start
    for i in range(nt):
        sz = base + (1 if i < rem else 0)
        tiles.append((t, sz))
        t += sz
    return tiles


def build_moe(counts):
    """Build + compile the 8-pass H-sliced expert MLP program.

    counts: per-expert token counts (same on every core; pass e covers
    exactly counts[e] tokens). Weight/x/g/y DRAM tensors hold the per-core
    slice data laid out expert-major (see moe_run for host layouts).
    """
    total = int(sum(counts))
    starts = np.concatenate([[0], np.cumsum(counts)]).astype(int)

    nc = bacc.Bacc("TRN2", target_bir_lowering=False, debug=False, num_devices=N_CORES)

    xt = nc.dram_tensor("xt", [D, total], F16, kind="ExternalInput")   # sorted x^T
    w1 = nc.dram_tensor("w1", [D, E * HS], F16, kind="ExternalInput")  # cols e*512..: this core's slice of expert e
    w2 = nc.dram_tensor("w2", [E * HS, D], F16, kind="ExternalInput")  # rows e*512..: this core's slice of expert e
    b1 = nc.dram_tensor("b1", [128, E * NHS], F32, kind="ExternalInput")
    g = nc.dram_tensor("g", [128, total], F32, kind="ExternalInput")   # gates, replicated rows
    yt = nc.dram_tensor("yt", [D, total], F16, kind="ExternalOutput")

    xt_ap, w1_ap, w2_ap, b1_ap, g_ap, yt_ap = (
        t.ap() for t in (xt, w1, w2, b1, g, yt)
    )

    with tile.TileContext(nc) as tc:
        with (
            tc.tile_pool(name="wpool", bufs=1) as wpool,
            tc.tile_pool(name="xpool", bufs=3) as xpool,
            tc.tile_pool(name="hpool", bufs=10) as hpool,
            tc.tile_pool(name="ypool", bufs=6) as ypool,
            tc.tile_pool(name="gpool", bufs=4) as gpool,
            tc.tile_pool(name="ph", bufs=4, space="PSUM") as ph_pool,
            tc.tile_pool(name="py", bufs=4, space="PSUM") as py_pool,
        ):
            def load_gate(t0, tn):
                g_sb = gpool.tile([128, 512], F32, name=f"gsb{t0}", tag="gsb")
                nc.sync.dma_start(g_sb[:, :tn], g_ap[:, t0:t0 + tn])
                return g_sb

            def load_tok_tile(t0, tn, split_first=False):
                # One DMA moves all 8 d-slices of this token tile into a wide
                # tile (d-slice j at columns [j*tn, (j+1)*tn)).
                xtile = xpool.tile([128, ND * 512], F16, name=f"xsb{t0}", tag="xsb")
                if split_first:
                    nc.sync.dma_start(xtile[:, :tn], xt_ap[0:128, t0:t0 + tn])
                    src = xt_ap[128:, t0:t0 + tn].rearrange("(dd p) t -> p dd t", p=128)
                    dst = xtile[:, tn:ND * tn].rearrange("p (dd t) -> p dd t", t=tn)
                    nc.sync.dma_start(dst, src)
                else:
                    src = xt_ap[:, t0:t0 + tn].rearrange("(dd p) t -> p dd t", p=128)
                    dst = xtile[:, :ND * tn].rearrange("p (dd t) -> p dd t", t=tn)
                    nc.sync.dma_start(dst, src)
                return [xtile[:, d * tn:(d + 1) * tn] for d in range(ND)]

            # PE warm-up: dummy matmuls on a zeroed tile cover the initial DMA
            # wait and un-throttle HAM before the real stream begins.
            warm = wpool.tile([128, 512], F16, name="warm", tag="warm")
            nc.vector.memset(warm[:], 0.0)
            warm_ps = ph_pool.tile([128, 512], F32, name="warmps", tag="ph")
            for _ in range(40):
                nc.tensor.matmul(warm_ps[:], warm[:, :128], warm[:], start=True, stop=True)

            pass_tiles = [_balanced_tiles(starts[e], counts[e]) for e in range(E)]

            # Prefetch the first TWO token tiles' x and gates before the bulk
            # weight DMAs: each pass burns through inputs ~8x faster per byte
            # than a whole-expert kernel, so the Sync trigger queue must not
            # put startup-critical tiles behind the 16-trigger w1 block.
            prefetched = {pass_tiles[0][0][0]: load_tok_tile(*pass_tiles[0][0], split_first=True)}
            g_prefetched = {pass_tiles[0][0][0]: load_gate(*pass_tiles[0][0])}
            b1_sb = wpool.tile([128, E * NHS], F32, name="b1sb", tag="b1sb")
            nc.sync.dma_start(b1_sb[:], b1_ap[:, :])
            t1 = pass_tiles[0][1][0]
            prefetched[t1] = load_tok_tile(*pass_tiles[0][1])
            g_prefetched[t1] = load_gate(*pass_tiles[0][1])

            # w1 slices, loaded in pass-consumption order as [128, 1024]
            # chunks (experts {2q, 2q+1} per chunk, 2KB DMA lines).
            w1_sb = [[None] * E for _ in range(ND)]  # [d][e] -> [128, HS]
            w1_dmas = [[] for _ in range(E // 2)]
            for q in range(E // 2):
                for d in range(ND):
                    t = wpool.tile([128, 2 * HS], F16, name=f"w1c{d}_{q}", tag=f"w1c{d}_{q}")
                    w1_dmas[q].append(nc.sync.dma_start(
                        t[:], w1_ap[d * 128:(d + 1) * 128, q * 2 * HS:(q + 1) * 2 * HS]
                    ))
                    w1_sb[d][2 * q] = t[:, :HS]
                    w1_sb[d][2 * q + 1] = t[:, HS:]

            # w2: one [128, 4*D] pack per expert on the idle Scalar queue,
            # dep-gated progressively (pack e released by an early pass-(e-1)
            # evac) so the 8.4MB stream doesn't contend with startup loads.
            w2_sb = []
            w2_dmas = []
            for e in range(E):
                t = wpool.tile([128, NHS * D], F16, name=f"w2p{e}", tag=f"w2p{e}")
                src = w2_ap[e * HS:(e + 1) * HS, :].rearrange("(ho p) d -> p ho d", p=128)
                dst = t.rearrange("p (ho d) -> p ho d", d=D)
                w2_dmas.append(nc.scalar.dma_start(dst, src))
                w2_sb.append(t)

            # All y-output DMAs on GpSimd so they never delay Sync's
            # startup/prefetch triggers.
            ydma_engines = [nc.gpsimd]
            n_y = 0

            for e in range(E):
                for ti, (t0, tn) in enumerate(pass_tiles[e]):
                    x_sb = prefetched.pop(t0) if t0 in prefetched else load_tok_tile(t0, tn)
                    g_sb = g_prefetched.pop(t0) if t0 in g_prefetched else load_gate(t0, tn)

                    # Layer 1: H-slice^T[j] = relu(sum_d W1s[d, j]^T X^T[d] + b1s[j])
                    h_sb = []
                    for j in range(NHS):
                        ph = ph_pool.tile([128, 512], F32, name=f"ph{e}_{t0}_{j}", tag="ph")
                        for d in range(ND):
                            nc.tensor.matmul(
                                ph[:, :tn],
                                w1_sb[d][e][:, j * 128:(j + 1) * 128],
                                x_sb[d][:, :tn],
                                start=(d == 0),
                                stop=(d == ND - 1),
                            )
                        ht = hpool.tile([128, 512], F16, name=f"hsb{e}_{t0}_{j}", tag="hsb")
                        evac = nc.vector.tensor_scalar(
                            ht[:, :tn], ph[:, :tn],
                            b1_sb[:, e * NHS + j:e * NHS + j + 1], 0.0,
                            op0=mybir.AluOpType.add, op1=mybir.AluOpType.max,
                        )
                        if ti == 0 and j == 0:
                            if e + 1 < E:
                                tile.add_dep_helper(w2_dmas[e + 1].ins, evac.ins, sync=True,
                                                    reason="w2 prefetch spread across passes")
                            # w1 chunk group q feeds passes 2q/2q+1; release it
                            # one pass-pair early so weight DMA bandwidth is
                            # spread across the run instead of the startup.
                            if e % 2 == 0 and e // 2 + 1 < E // 2:
                                for wd in w1_dmas[e // 2 + 1]:
                                    tile.add_dep_helper(wd.ins, evac.ins, sync=True,
                                                        reason="w1 prefetch spread across passes")
                        h_sb.append(ht)

                    # Layer 2: Y^T[do] += g * sum_j W2s[j, do]^T Hs^T[j]
                    for do in range(ND):
                        py = py_pool.tile([128, 512], F32, name=f"py{e}_{t0}_{do}", tag="py")
                        for j in range(NHS):
                            nc.tensor.matmul(
                                py[:, :tn],
                                w2_sb[e][:, j * D + do * 128:j * D + (do + 1) * 128],
                                h_sb[j][:, :tn],
                                start=(j == 0),
                                stop=(j == NHS - 1),
                            )
                        y_sb = ypool.tile([128, 512], F16, name=f"ysb{e}_{t0}_{do}", tag="ysb")
                        nc.vector.tensor_mul(y_sb[:, :tn], py[:, :tn], g_sb[:, :tn])
                        eng = ydma_engines[n_y % len(ydma_engines)]
                        n_y += 1
                        eng.dma_start(yt_ap[do * 128:(do + 1) * 128, t0:t0 + tn], y_sb[:, :tn])

    nc.compile()
    return nc


def _route(x, wg, bg):
    """Host router in fp64: per-token top-2 experts and softmax gates."""
    logits = x.astype(np.float64) @ wg.astype(np.float64).T + bg.astype(np.float64)
    top2 = np.argpartition(-logits, 1, axis=1)[:, :TOP_K]  # two largest, unordered
    vals = np.take_along_axis(logits, top2, axis=1)
    ex = np.exp(vals - vals.max(axis=1, keepdims=True))
    gates = ex / ex.sum(axis=1, keepdims=True)
    idxs, gs = [], []
    for e in range(E):
        mask = top2 == e
        rows = np.nonzero(mask.any(axis=1))[0]
        idxs.append(rows)
        gs.append(gates[mask].astype(np.float32))
    return idxs, gs


def moe_run(x, wg, bg, w1, b1, w2, b2, trace=False, trace_kwargs=None):
    x = np.ascontiguousarray(np.asarray(x, np.float32))
    wg = np.asarray(wg, np.float32)
    bg = np.asarray(bg, np.float32)
    w1 = np.asarray(w1, np.float32)
    b1 = np.asarray(b1, np.float32)
    w2 = np.asarray(w2, np.float32)
    b2 = np.asarray(b2, np.float32)
    B = x.shape[0]

    idxs, gs = _route(x, wg, bg)
    counts = [len(r) for r in idxs]
    total = sum(counts)

    nc = build_moe(counts)

    # Shared (identical on every core): sorted activations and gates.
    order = np.concatenate(idxs)
    xt_all = np.ascontiguousarray(x[order].T).astype(np.float16)       # (D, total)
    g_all = np.concatenate(gs).astype(np.float32)                      # (total,)
    g_rep = np.ascontiguousarray(np.broadcast_to(g_all, (128, total)))

    in_maps = []
    for c in range(N_CORES):
        # Core c's H-slice [c*512, (c+1)*512) of every expert.
        w1c = np.concatenate([w1[e][:, c * HS:(c + 1) * HS] for e in range(E)], axis=1)
        w2c = np.concatenate([w2[e][c * HS:(c + 1) * HS, :] for e in range(E)], axis=0)
        b1c = np.concatenate([b1[e][c * HS:(c + 1) * HS].reshape(NHS, 128).T
                              for e in range(E)], axis=1)
        in_maps.append({
            "xt": xt_all,
            "w1": w1c.astype(np.float16),
            "w2": w2c.astype(np.float16),
            "b1": np.ascontiguousarray(b1c),
            "g": g_rep,
        })

    kwargs = {}
    if trace:
        kwargs["trace"] = True
        if trace_kwargs:
            kwargs.update(trace_kwargs)
    res = run_bass_kernel_spmd(nc, in_maps, core_ids=list(range(N_CORES)), **kwargs)

    # Sum the 8 cores' H-slice partials, then scatter-add per-expert segments.
    ysum = res.results[0]["yt"].astype(np.float32)
    for c in range(1, N_CORES):
        ysum += res.results[c]["yt"].astype(np.float32)

    out = np.zeros((B, D), np.float32)
    t = 0
    for e in range(E):
        n = counts[e]
        out[idxs[e]] += ysum[:, t:t + n].T + gs[e][:, None] * b2[e][None, :]
        t += n
    return out, res


def kernel(x, wg, bg, w1, b1, w2, b2):
    out, _ = moe_run(x, wg, bg, w1, b1, w2, b2, trace=False)
    return out



# revision 2
# speedup vs baseline: 1.6417x; 1.6417x over previous
"""MoE layer (top-2 routing, 8 experts) on 8 Trainium2 NeuronCores.

Strategy — expert-parallel with hidden-dim (H) slicing for perfect balance:
  - Host computes the gate (router math in fp64 numpy): logits, top-2 experts
    per token, softmax gates; tokens are sorted into per-expert segments.
  - ReLU is elementwise in H, so each expert MLP decomposes exactly into 8
    independent H-slice MLPs (D x 512 x D). Core c holds slice c of EVERY
    expert (same 16.8MB fp16 weight footprint as one whole expert).
  - The kernel runs 8 passes; pass e = all 8 cores compute expert e's slice
    over exactly n_e tokens (identical shapes on every core -> SPMD, zero
    padding, perfect load balance).
  - Each core emits gate-weighted partial outputs; host sums the 8 cores'
    partials and scatter-adds each token's two expert contributions.

Schedule notes (from profile analysis): the PE stream is gap-free mid-run, so
the only recoverable time is at the two ends. Startup: x tiles (sync queue),
w1 chunks (scalar queue) and gates/b1/w2-pack0 (gpsimd queue) load in
parallel, with the first pass's tiles ramped 128/256/384 so real matmuls
start as soon as the first x/w1 pieces land (~9us) instead of waiting for a
bulk prefetch. Tail: y DMAs alternate sync/gpsimd so neither queue builds a
drain backlog, and the final tile is 128 tokens so the post-matmul chain is
short.

Hardcoded problem shape: x(8192,1024) w1(8,1024,4096) w2(8,4096,1024).
"""

import numpy as np

import concourse.tile as tile
import concourse.mybir as mybir
from concourse import bacc
from concourse.bass_utils import run_bass_kernel_spmd

E = 8          # experts
D = 1024       # model dim
H = 4096       # hidden dim
HS = H // 8    # per-core hidden slice (512)
NHS = HS // 128  # h-tiles per slice (4)
TOP_K = 2
N_CORES = 8
ND = D // 128   # 8 d-tiles

F32 = mybir.dt.float32
F16 = mybir.dt.float16


def _balanced(n, max_tile=512):
    """Near-equal split of n into ceil(n/max_tile) tiles."""
    nt = max(1, -(-n // max_tile))
    base, rem = divmod(n, nt)
    return [base + (1 if i < rem else 0) for i in range(nt)]


def _pass_sizes(n, first=False, last=False):
    if first and n >= 1280:
        return [128, 256, 384] + _balanced(n - 768)
    if last and n >= 768:
        return _balanced(n - 128) + [128]
    return _balanced(n)


def build_moe(counts):
    """Build + compile the 8-pass H-sliced expert MLP program.

    counts: per-expert token counts (same on every core; pass e covers
    exactly counts[e] tokens). Weight/x/g/y DRAM tensors hold the per-core
    slice data laid out expert-major (see moe_run for host layouts).
    """
    total = int(sum(counts))
    starts = np.concatenate([[0], np.cumsum(counts)]).astype(int)

    nc = bacc.Bacc("TRN2", target_bir_lowering=False, debug=False, num_devices=N_CORES)

    xt = nc.dram_tensor("xt", [D, total], F16, kind="ExternalInput")   # sorted x^T
    w1 = nc.dram_tensor("w1", [D, E * HS], F16, kind="ExternalInput")  # cols e*512..: this core's slice of expert e
    w2 = nc.dram_tensor("w2", [E * HS, D], F16, kind="ExternalInput")  # rows e*512..: this core's slice of expert e
    b1 = nc.dram_tensor("b1", [128, E * NHS], F32, kind="ExternalInput")
    g = nc.dram_tensor("g", [128, total], F16, kind="ExternalInput")   # gates, replicated rows
    yt = nc.dram_tensor("yt", [D, total], F16, kind="ExternalOutput")

    xt_ap, w1_ap, w2_ap, b1_ap, g_ap, yt_ap = (
        t.ap() for t in (xt, w1, w2, b1, g, yt)
    )

    pass_tiles = []
    for e in range(E):
        szs = _pass_sizes(int(counts[e]), first=(e == 0), last=(e == E - 1))
        t0 = int(starts[e])
        tl = []
        for s in szs:
            tl.append((t0, s))
            t0 += s
        pass_tiles.append(tl)

    with tile.TileContext(nc) as tc:
        with (
            tc.tile_pool(name="wpool", bufs=1) as wpool,
            tc.tile_pool(name="xpool", bufs=3) as xpool,
            tc.tile_pool(name="hpool", bufs=10) as hpool,
            tc.tile_pool(name="ypool", bufs=6) as ypool,
            tc.tile_pool(name="gpool", bufs=4) as gpool,
            tc.tile_pool(name="ph", bufs=4, space="PSUM") as ph_pool,
            tc.tile_pool(name="py", bufs=4, space="PSUM") as py_pool,
        ):
            def load_gate(t0, tn, eng=None):
                g_sb = gpool.tile([128, 512], F16, name=f"gsb{t0}", tag="gsb")
                (eng or nc.gpsimd).dma_start(g_sb[:, :tn], g_ap[:, t0:t0 + tn])
                return g_sb

            def load_tok_tile(t0, tn, granular=False):
                # d-slice j lives at columns [j*tn, (j+1)*tn) of a wide tile.
                xtile = xpool.tile([128, ND * 512], F16, name=f"xsb{t0}", tag="xsb")
                if granular:
                    # Paired per-d DMAs so the first d-chunks land (and the
                    # first matmuls fire) before the whole tile transfers.
                    for q in range(ND // 2):
                        src = xt_ap[q * 256:(q + 1) * 256, t0:t0 + tn].rearrange(
                            "(dd p) t -> p dd t", p=128)
                        dst = xtile[:, 2 * q * tn:(2 * q + 2) * tn].rearrange(
                            "p (dd t) -> p dd t", t=tn)
                        nc.sync.dma_start(dst, src)
                else:
                    src = xt_ap[:, t0:t0 + tn].rearrange("(dd p) t -> p dd t", p=128)
                    dst = xtile[:, :ND * tn].rearrange("p (dd t) -> p dd t", t=tn)
                    nc.sync.dma_start(dst, src)
                return [xtile[:, d * tn:(d + 1) * tn] for d in range(ND)]

            # Startup prefetch, three queues in parallel:
            #   sync:   x tile0 (granular) + tile1
            #   gpsimd: g0, g1, b1, w2 pack0
            #   scalar: w1 chunks q0 (then gated q1..q3 / w2 packs 1..7)
            prefetched = {pass_tiles[0][0][0]: load_tok_tile(*pass_tiles[0][0], granular=True)}
            g_prefetched = {pass_tiles[0][0][0]: load_gate(*pass_tiles[0][0])}
            t1 = pass_tiles[0][1][0]
            prefetched[t1] = load_tok_tile(*pass_tiles[0][1])
            g_prefetched[t1] = load_gate(*pass_tiles[0][1])
            b1_sb = wpool.tile([128, E * NHS], F32, name="b1sb", tag="b1sb")
            nc.gpsimd.dma_start(b1_sb[:], b1_ap[:, :])

            # w2: one [128, 4*D] pack per expert. Pack 0 rides the gpsimd
            # queue (parallel with scalar's w1 q0 stream) so tile0's layer 2
            # is never weight-gated; packs 1..7 go on scalar, dep-gated
            # progressively (pack e released by an early pass-(e-1) evac).
            w2_sb = [None] * E
            w2_dmas = [None] * E

            def load_w2(e, eng):
                t = wpool.tile([128, NHS * D], F16, name=f"w2p{e}", tag=f"w2p{e}")
                src = w2_ap[e * HS:(e + 1) * HS, :].rearrange("(ho p) d -> p ho d", p=128)
                dst = t.rearrange("p (ho d) -> p ho d", d=D)
                w2_dmas[e] = eng.dma_start(dst, src)
                w2_sb[e] = t

            load_w2(0, nc.gpsimd)

            # w1 slices on scalar, in pass-consumption order as [128, 1024]
            # chunks (experts {2q, 2q+1} per chunk, 2KB DMA lines). The w2
            # packs interleave in release order to keep the FIFO consistent:
            # q0*8, p1, p2, q1*8, p3, p4, q2*8, p5, p6, q3*8, p7.
            w1_sb = [[None] * E for _ in range(ND)]  # [d][e] -> [128, HS]
            w1_dmas = [[] for _ in range(E // 2)]

            def load_w1_group(q):
                for d in range(ND):
                    t = wpool.tile([128, 2 * HS], F16, name=f"w1c{d}_{q}", tag=f"w1c{d}_{q}")
                    w1_dmas[q].append(nc.scalar.dma_start(
                        t[:], w1_ap[d * 128:(d + 1) * 128, q * 2 * HS:(q + 1) * 2 * HS]
                    ))
                    w1_sb[d][2 * q] = t[:, :HS]
                    w1_sb[d][2 * q + 1] = t[:, HS:]

            load_w1_group(0)
            load_w2(1, nc.scalar)
            load_w2(2, nc.scalar)
            load_w1_group(1)
            load_w2(3, nc.scalar)
            load_w2(4, nc.scalar)
            load_w1_group(2)
            load_w2(5, nc.scalar)
            load_w2(6, nc.scalar)
            load_w1_group(3)
            load_w2(7, nc.scalar)

            # y-output DMAs alternate sync/gpsimd so no single queue builds
            # a drain backlog at the end of the run.
            ydma_engines = [nc.sync, nc.gpsimd]
            n_y = 0

            for e in range(E):
                for ti, (t0, tn) in enumerate(pass_tiles[e]):
                    x_sb = prefetched.pop(t0) if t0 in prefetched else load_tok_tile(t0, tn)
                    g_sb = g_prefetched.pop(t0) if t0 in g_prefetched else load_gate(t0, tn)

                    # Layer 1: H-slice^T[j] = relu(sum_d W1s[d, j]^T X^T[d] + b1s[j])
                    h_sb = []
                    for j in range(NHS):
                        ph = ph_pool.tile([128, 512], F32, name=f"ph{e}_{t0}_{j}", tag="ph")
                        for d in range(ND):
                            nc.tensor.matmul(
                                ph[:, :tn],
                                w1_sb[d][e][:, j * 128:(j + 1) * 128],
                                x_sb[d][:, :tn],
                                start=(d == 0),
                                stop=(d == ND - 1),
                            )
                        ht = hpool.tile([128, 512], F16, name=f"hsb{e}_{t0}_{j}", tag="hsb")
                        evac = nc.vector.tensor_scalar(
                            ht[:, :tn], ph[:, :tn],
                            b1_sb[:, e * NHS + j:e * NHS + j + 1], 0.0,
                            op0=mybir.AluOpType.add, op1=mybir.AluOpType.max,
                        )
                        if ti == 0 and j == 0:
                            if e + 1 < E:
                                tile.add_dep_helper(w2_dmas[e + 1].ins, evac.ins, sync=True,
                                                    reason="w2 prefetch spread across passes")
                            # w1 chunk group q feeds passes 2q/2q+1; release it
                            # one pass-pair early so weight DMA bandwidth is
                            # spread across the run instead of the startup.
                            if e % 2 == 0 and e // 2 + 1 < E // 2:
                                for wd in w1_dmas[e // 2 + 1]:
                                    tile.add_dep_helper(wd.ins, evac.ins, sync=True,
                                                        reason="w1 prefetch spread across passes")
                        h_sb.append(ht)

                    # Layer 2: Y^T[do] += g * sum_j W2s[j, do]^T Hs^T[j]
                    for do in range(ND):
                        py = py_pool.tile([128, 512], F32, name=f"py{e}_{t0}_{do}", tag="py")
                        for j in range(NHS):
                            nc.tensor.matmul(
                                py[:, :tn],
                                w2_sb[e][:, j * D + do * 128:j * D + (do + 1) * 128],
                                h_sb[j][:, :tn],
                                start=(j == 0),
                                stop=(j == NHS - 1),
                            )
                        y_sb = ypool.tile([128, 512], F16, name=f"ysb{e}_{t0}_{do}", tag="ysb")
                        nc.vector.tensor_mul(y_sb[:, :tn], py[:, :tn], g_sb[:, :tn])
                        eng = ydma_engines[n_y % len(ydma_engines)]
                        n_y += 1
                        eng.dma_start(yt_ap[do * 128:(do + 1) * 128, t0:t0 + tn], y_sb[:, :tn])

    nc.compile()
    return nc


def _route(x, wg, bg):
    """Host router in fp64: per-token top-2 experts and softmax gates."""
    logits = x.astype(np.float64) @ wg.astype(np.float64).T + bg.astype(np.float64)
    top2 = np.argpartition(-logits, 1, axis=1)[:, :TOP_K]  # two largest, unordered
    vals = np.take_along_axis(logits, top2, axis=1)
    ex = np.exp(vals - vals.max(axis=1, keepdims=True))
    gates = ex / ex.sum(axis=1, keepdims=True)
    idxs, gs = [], []
    for e in range(E):
        mask = top2 == e
        rows = np.nonzero(mask.any(axis=1))[0]
        idxs.append(rows)
        gs.append(gates[mask].astype(np.float32))
    return idxs, gs


def moe_run(x, wg, bg, w1, b1, w2, b2, trace=False, trace_kwargs=None):
    x = np.ascontiguousarray(np.asarray(x, np.float32))
    wg = np.asarray(wg, np.float32)
    bg = np.asarray(bg, np.float32)
    w1 = np.asarray(w1, np.float32)
    b1 = np.asarray(b1, np.float32)
    w2 = np.asarray(w2, np.float32)
    b2 = np.asarray(b2, np.float32)
    B = x.shape[0]

    idxs, gs = _route(x, wg, bg)
    counts = [len(r) for r in idxs]
    total = sum(counts)

    nc = build_moe(counts)

    # Shared (identical on every core): sorted activations and gates.
    order = np.concatenate(idxs)
    xt_all = np.ascontiguousarray(x[order].T).astype(np.float16)       # (D, total)
    g_all = np.concatenate(gs).astype(np.float16)                      # (total,)
    g_rep = np.ascontiguousarray(np.broadcast_to(g_all, (128, total)))

    in_maps = []
    for c in range(N_CORES):
        # Core c's H-slice [c*512, (c+1)*512) of every expert.
        w1c = np.concatenate([w1[e][:, c * HS:(c + 1) * HS] for e in range(E)], axis=1)
        w2c = np.concatenate([w2[e][c * HS:(c + 1) * HS, :] for e in range(E)], axis=0)
        b1c = np.concatenate([b1[e][c * HS:(c + 1) * HS].reshape(NHS, 128).T
                              for e in range(E)], axis=1)
        in_maps.append({
            "xt": xt_all,
            "w1": w1c.astype(np.float16),
            "w2": w2c.astype(np.float16),
            "b1": np.ascontiguousarray(b1c),
            "g": g_rep,
        })

    kwargs = {}
    if trace:
        kwargs["trace"] = True
        if trace_kwargs:
            kwargs.update(trace_kwargs)
    res = run_bass_kernel_spmd(nc, in_maps, core_ids=list(range(N_CORES)), **kwargs)

    # Sum the 8 cores' H-slice partials, then scatter-add per-expert segments.
    ysum = res.results[0]["yt"].astype(np.float32)
    for c in range(1, N_CORES):
        ysum += res.results[c]["yt"].astype(np.float32)

    out = np.zeros((B, D), np.float32)
    t = 0
    for e in range(E):
        n = counts[e]
        out[idxs[e]] += ysum[:, t:t + n].T + gs[e][:, None] * b2[e][None, :]
        t += n
    return out, res


def kernel(x, wg, bg, w1, b1, w2, b2):
    out, _ = moe_run(x, wg, bg, w1, b1, w2, b2, trace=False)
    return out
